# revision 1
# baseline (speedup 1.0000x reference)
"""Trainium2 Bass kernel for nn_Attention_28750511080014 (sparse/GQA attention).

Reference semantics (replicated exactly, including the noncanonical plain
reshape):
  qkv = x @ w_attn.T                         [B,S,1536]
  q = qkv[..., :1024].reshape(B, 16, S, 64)  # plain reshape, no transpose
  k = qkv[..., 1024:1280].reshape(B, 4, S, 64); v likewise
  causal softmax attention with repeat_interleave(4) on kv heads
  y -> transpose -> [B,S,1024] @ w_proj.T

Key structural fact: the plain reshape makes q-head n's [2048, 64] matrix a
contiguous reinterpretation of rows [n*128,(n+1)*128) of the [2048,1024]
q2d = x @ wq.T matrix; kv-head m similarly uses rows [m*512,(m+1)*512) of
the [2048,256] k2d/v2d matrices. So q-heads {4g..4g+3} and kv-head g only
need x rows [g*512,(g+1)*512).

Sharding: 8 cores = 2 batches x 4 kv-groups. Core (b,g) computes 4 q-heads +
1 kv head and a partial output projection over its 256 y2d columns; host sums
the 4 partials per batch (row-parallel linear unshard).

On-device layout is fully transposed (k on partitions) so PV needs no
transposes: S_T[k,q] = K.T-tile.T @ Q.T, P_T = exp(S_T) (no max subtraction
needed: scores are O(1) by construction), Y_T[(d|ones),q] accumulates
V1.T @ P_T over k-tiles giving the softmax denominator for free in row 64.
Normalization: reciprocal of the den row, gpsimd partition_broadcast across
64 partitions, one DVE multiply into y2dT. Projection: out.T = wpT.T @ y2dT,
emitted one quarter late so it fills PE slack inside the next quarter.
Matmuls use float32r (full PE rate at N=512); P/V1/masks are bf16; S_T
matmul pairs pack two heads onto disjoint PE row-groups via base partitions.
"""

import sys
import numpy as np
from contextlib import ExitStack

for _p in ("/opt/trn_rl_repo",):
    if _p not in sys.path:
        sys.path.insert(0, _p)

B, S, H = 2, 2048, 1024
NQ, NKV, HD = 16, 4, 64
GHD = 256          # kv projection width (4 heads * 64)
G = 4              # q heads per kv head == cores per batch
SB = 512           # x rows per core block
W_QKV = H + 2 * GHD  # 1536
NCORES = 8
NH = 4             # local q heads per core
KT = 16            # k-tiles of 128 over S
SCALE = 0.125      # 1/sqrt(64), folded into wq on host

_NC = None


def _build_body(ctx, tc, xT, wT, wpT, msk, mskb, outT):
    import concourse.bass as bass
    import concourse.mybir as mybir

    nc = tc.nc
    dt = mybir.dt
    f32 = dt.float32
    f32r = dt.float32r
    bf16 = dt.bfloat16
    Exp = mybir.ActivationFunctionType.Exp

    # ---- pools ----
    cpool = ctx.enter_context(tc.tile_pool(name="consts", bufs=1))
    inp = ctx.enter_context(tc.tile_pool(name="inputs", bufs=1))
    psA = ctx.enter_context(tc.tile_pool(name="psA", bufs=2, space="PSUM"))
    psB = ctx.enter_context(tc.tile_pool(name="psB", bufs=4, space="PSUM"))
    ptp = ctx.enter_context(tc.tile_pool(name="ptp", bufs=6))
    ptq = ctx.enter_context(tc.tile_pool(name="ptq", bufs=4))

    # ---- SBUF input tensors ----
    xT_sb = inp.tile([128, 8, SB], f32r, tag="xt")
    wT_sb = inp.tile([128, 8, W_QKV], f32r, tag="wt")
    wpT_sb = inp.tile([128, 2, H], f32r, tag="wpt")
    mask_sb = cpool.tile([128, 4, 512], bf16, tag="mask")
    idn_sb = cpool.tile([64, 64], f32r, tag="idn")

    nc.sync.dma_start(idn_sb[:, :], msk[0:64, :])
    for ht in range(8):
        nc.sync.dma_start(xT_sb[:, ht, :], xT[ht * 128:(ht + 1) * 128, :])
    for ht in range(8):
        nc.sync.dma_start(wT_sb[:, ht, 1024:1536],
                          wT[ht * 128:(ht + 1) * 128, 1024:1536])
    nc.sync.dma_start(mask_sb[:, :, :], mskb[:, :])
    for ht in range(8):
        nc.sync.dma_start(wT_sb[:, ht, 0:512],
                          wT[ht * 128:(ht + 1) * 128, 0:512])
    for ht in range(8):
        nc.sync.dma_start(wT_sb[:, ht, 512:1024],
                          wT[ht * 128:(ht + 1) * 128, 512:1024])
    nc.sync.dma_start(wpT_sb[:, 0, :], wpT[0:128, :])
    nc.sync.dma_start(wpT_sb[:, 1, :], wpT[128:256, :])

    # ---- qkvT SBUF tensors ----
    # qT3d[64*(j%2)+d, c, 128*j + r'] = q_slab.T[c*64+d, 128*j+r']
    # head j = 2p+m lives at partition half m = j%2; s2 = 16*r_local + c
    qT3d = cpool.tile([128, 16, SB], f32r, tag="qT")
    kTs = cpool.tile([128, S], f32r, tag="kT")   # s2-ordered, dup halves
    vTs = cpool.tile([64, S], f32r, tag="vT")    # s2-ordered
    v1_sb = cpool.tile([128, KT, HD + 1], bf16, tag="v1")
    y2dT = cpool.tile([128, 2, S], f32r, tag="y2dT")
    rr_sb = cpool.tile([1, 512], f32, tag="rr")

    nc.vector.memset(v1_sb[:, :, HD:HD + 1], 1.0)

    def s2_ap(t3, mlo, nc_, r0, nr):
        """[64, nr, nc_] AP over t3 partition half mlo, s2-ordered."""
        return t3[64 * mlo:64 * mlo + 64, 0:nc_, r0:r0 + nr].rearrange(
            "d c r -> d r c")

    # ---- QKV projection ----
    for oc in (8, 9, 10, 11, 0, 1, 2, 3, 4, 5, 6, 7):
        ps = psB.tile([128, 512], f32, tag="yt")
        for ht in range(8):
            nc.tensor.matmul(
                ps[:, 0:SB],
                wT_sb[:, ht, oc * 128:(oc + 1) * 128],
                xT_sb[:, ht, :],
                start=(ht == 0), stop=(ht == 7),
            )
        for half in range(2):
            src = ps[half * 64:(half + 1) * 64, 0:SB]
            if oc < 8:
                c = 2 * oc + half
                # heads j in {m, m+2} per copy: [64, 2, 128] block APs
                s3 = src.rearrange("p (b z) -> p b z", z=128)
                d3 = qT3d[:, c, :].rearrange("p (b z) -> p b z", z=128)
                for m in range(2):
                    sap = s3[:, m:m + 3:2, :]
                    dap = d3[64 * m:64 * m + 64, m:m + 3:2, :]
                    if (half + m) % 2 == 0:
                        nc.vector.tensor_copy(out=dap, in_=sap)
                    else:
                        nc.scalar.copy(dap, sap)
            elif oc < 10:
                c = 2 * (oc - 8) + half
                nc.vector.tensor_copy(out=kTs[0:64, c:S:4], in_=src)
                nc.scalar.copy(kTs[64:128, c:S:4], src)
            else:
                c = 2 * (oc - 10) + half
                nc.vector.tensor_copy(out=vTs[0:64, c:S:4], in_=src)
        if oc == 11:
            # V transposes fill the PE gap while q-column DMAs land
            for kt in range(KT):
                tp = psA.tile([128, 1024], f32r, tag="st")
                with nc.allow_low_precision(reason="transpose is data movement"):
                    nc.tensor.transpose(
                        tp[0:128, 0:64],
                        vTs[0:64, 128 * kt:128 * (kt + 1)],
                        idn_sb[0:64, 0:64],
                    )
                nc.vector.tensor_copy(out=v1_sb[:, kt, 0:HD],
                                      in_=tp[0:128, 0:64])

    # ---- attention: quarter-halves h' of 512 q, h outer for proj overlap ----
    def proj_pieces(hq):
        Q0q = 512 * hq

        def piece(ot):
            def go():
                ctx2 = tc.high_priority(offset=-120)
                ctx2.__enter__()
                pp = psB.tile([128, 512], f32, tag="yt")
                for ct in range(2):
                    nc.tensor.matmul(
                        pp[:, 0:512],
                        wpT_sb[:, ct, ot * 128:(ot + 1) * 128],
                        y2dT[:, ct, Q0q:Q0q + 512],
                        start=(ct == 0), stop=(ct == 1),
                    )
                ob = ptp.tile([128, 1024], f32, tag="ob")
                nc.vector.tensor_copy(out=ob[:, 0:512], in_=pp[:, 0:512])
                nc.sync.dma_start(
                    outT[ot * 128:(ot + 1) * 128, Q0q:Q0q + 512],
                    ob[:, 0:512],
                )
                ctx2.__exit__(None, None, None)
            return go
        return [piece(ot) for ot in range(8)]

    def emit_norm(p, h):
        Q0n = 512 * h
        for m in range(2):
            j = 2 * p + m
            yt = norm_pend[(p, h)][m]
            nc.vector.reciprocal(rr_sb[0:1, 0:512], yt[64:65, 0:512])
            rbs = ptp.tile([128, 1024], f32, tag="rbs")
            nc.gpsimd.partition_broadcast(
                rbs[0:64, 0:512], rr_sb[0:1, 0:512], channels=64)
            nc.vector.tensor_mul(
                y2dT[64 * m:64 * m + 64, j // 2, Q0n:Q0n + 512],
                yt[0:64, 0:512],
                rbs[0:64, 0:512],
            )

    norm_pend = {}
    pending = []   # deferred work closures, drained one per kt unit
    for h in range(4):
        Q0 = 512 * h
        kt_max = 4 * h + 3
        for p in range(2):
            ytA = psB.tile([65, 512], f32, tag="yt")   # head 2p
            ytB = psB.tile([65, 512], f32, tag="yt")   # head 2p+1
            yts = (ytA, ytB)
            norm_pend[(p, h)] = yts
            pend = None

            for kt in range(kt_max + 1):
                st = psA.tile([128, 1024], f32, tag="st")
                for m in range(2):
                    j = 2 * p + m
                    nc.tensor.matmul(
                        st[:, 512 * m:512 * m + 512],
                        kTs[64 * m:64 * m + 64, 128 * kt:128 * (kt + 1)],
                        s2_ap(qT3d, m, 16, 128 * j + 32 * h, 32),
                        start=True, stop=True,
                    )
                pt = ptq.tile([128, 1024], bf16, tag="pt")
                nc.scalar.activation(pt[:, :], st[:, :], Exp)
                if kt >= 4 * h:  # diagonal: mask invalid wedge
                    mm = kt - 4 * h
                    nc.vector.tensor_mul(
                        pt[:, 0:512], pt[:, 0:512], mask_sb[:, mm, :])
                    nc.vector.tensor_mul(
                        pt[:, 512:1024], pt[:, 512:1024], mask_sb[:, mm, :])
                if pending:
                    pending.pop(0)()
                if pend is not None:
                    _emit_pv(nc, v1_sb, pend)
                pend = (yts, kt, pt, kt_max)
            _emit_pv(nc, v1_sb, pend)

            # defer this (p,h)'s norm: emit previous pair's now
            prev = (1, h - 1) if p == 0 else (0, h)
            if prev in norm_pend:
                emit_norm(*prev)
                del norm_pend[prev]
            if p == 0 and h == 3:
                pending.extend(proj_pieces(2))
        if 0 < h < 3:
            pending.extend(proj_pieces(h - 1))
    # drain: last norms, remaining proj pieces, final quarter proj
    emit_norm(1, 3)
    for go in pending:
        go()
    for go in proj_pieces(3):
        go()


def _emit_pv(nc, v1_sb, pend):
    yts, kt, pt, kt_max = pend
    for m in range(2):
        nc.tensor.matmul(
            yts[m][0:65, 0:512],
            v1_sb[:, kt, :],
            pt[:, 512 * m:512 * m + 512],
            start=(kt == 0), stop=(kt == kt_max),
        )


def _build():
    import concourse.tile as tile
    from concourse import bacc
    import concourse.mybir as mybir

    dt = mybir.dt
    nc = bacc.Bacc("TRN2", target_bir_lowering=False, debug=False,
                   num_devices=NCORES)
    xT = nc.dram_tensor("xt", [H, SB], dt.float32r, kind="ExternalInput").ap()
    wT = nc.dram_tensor("wt", [H, W_QKV], dt.float32r, kind="ExternalInput").ap()
    wpT = nc.dram_tensor("wpt", [GHD, H], dt.float32r, kind="ExternalInput").ap()
    msk = nc.dram_tensor("msk", [64, 64], dt.float32r,
                         kind="ExternalInput").ap()
    mskb = nc.dram_tensor("mskb", [128, 2048], dt.bfloat16,
                          kind="ExternalInput").ap()
    outT = nc.dram_tensor("outt", [H, S], dt.float32,
                          kind="ExternalOutput").ap()

    with tile.TileContext(nc) as tc, ExitStack() as ctx:
        ctx.enter_context(
            nc.allow_low_precision(reason="f32r rounding is intentional"))
        _build_body(ctx, tc, xT, wT, wpT, msk, mskb, outT)
    nc.compile()
    return nc


def _get_nc():
    global _NC
    if _NC is None:
        _NC = _build()
    return _NC


def _host_inputs(x, w_attn, w_proj):
    import ml_dtypes
    x = np.asarray(x, np.float32)
    w_attn = np.asarray(w_attn, np.float32)
    w_proj = np.asarray(w_proj, np.float32)
    wq = w_attn[:H] * SCALE
    wT_np = np.ascontiguousarray(
        np.concatenate([wq, w_attn[H:]], axis=0).T)          # [1024, 1536]

    msk = np.ascontiguousarray(np.eye(64, dtype=np.float32))
    mskb = np.zeros((128, 2048), np.float32)
    k_idx = np.arange(128)[:, None]
    qq = np.arange(512)[None, :]
    for m in range(4):
        mskb[:, m * 512:(m + 1) * 512] = (k_idx <= qq - 128 * m)
    mskb = mskb.astype(ml_dtypes.bfloat16)

    in_maps = []
    for c in range(NCORES):
        b, g = c // 4, c % 4
        xT = np.ascontiguousarray(x[b, g * SB:(g + 1) * SB, :].T)
        wpT = np.ascontiguousarray(w_proj[:, g * GHD:(g + 1) * GHD].T)
        in_maps.append({"xt": xT, "wt": wT_np, "wpt": wpT,
                        "msk": msk, "mskb": mskb})
    return in_maps


def _gather(results):
    out = np.zeros((B, S, H), np.float32)
    for c in range(NCORES):
        b = c // 4
        out[b] += results[c]["outt"].T
    return out


def kernel(x, w_attn, w_proj):
    from concourse.bass_utils import run_bass_kernel_spmd
    nc = _get_nc()
    in_maps = _host_inputs(x, w_attn, w_proj)
    res = run_bass_kernel_spmd(nc, in_maps, core_ids=list(range(NCORES)))
    return _gather(res.results)

